# revision 13
# baseline (speedup 1.0000x reference)
"""Trainium2 Bass kernel for nn_Ensemble_FC (BatchEnsemble fully-connected layer).

Math (reference):
    emb   = relu(alpha @ enc1_w.T + enc1_b)          # (M, H)
    mu    = emb @ encm_w.T + encm_b                  # (M, H)
    z     = eps * exp(0.5 * mu) + mu
    adec  = z @ dec_w.T + dec_b                      # (M, IN)
    out[m*B+i, o] = (sum_k x[i,k] * adec[m,k] * fc_w[o,k]) * gamma[m,o] + bias_p[m,o]

Sharding: tensor-parallel column-split of fc_w / gamma / bias_p over
out_features (4096 -> 8 x 512).  Every core computes the tiny encoder
redundantly and the full (M*B = 2048)-row GEMM for its 512 output columns.

On-chip layout is transposed ([feature, row]) so per-model scales
(adec, gamma, bias) are per-partition scalars:
    out_core[o_local, m*B+i] = psum * gamma + bias,
    psum = sum_kc  wT[kc, o-chunk].T @ (xT[kc] * adecT[kc, m])
GEMM and encoder weights run in bf16 (rounded on host), fp32 PSUM
accumulation; the mu/exp/z chain and all epilogues are fp32.

Perf structure (trace-driven):
- ~7us fixed prologue (barriers + IRAM loads) before any instruction runs.
- Encoder constants go in small bf16 packs at the head of the ACT HWDGE
  ring so they don't queue behind the 8 MB bulk stream; gamma/bias ride a
  separate tiny f32 pack.  x stream on the SP ring, w stream on the Pool
  SWDGE ring — descriptor generation is ~0.6-1.0us of sequencer time per
  dma_start, so the three queues are issued in parallel.
- PE warm-up matmuls bridge the const-DMA latency and trip the HAM clock
  gate (cold PE runs at 1.2 GHz; >3.4us of sustained work => 2.4 GHz).
- dec_b is folded into the adec matmul via an augmented ones row; all 32
  adec chunk matmuls land in one PSUM tile, copied out once.
- The last model runs oc-major over pre-materialized scaled activations
  so the final epilogue + store overlaps the matmul stream.
"""

import os
import sys

for _p in ("/opt/trn_rl_repo",):
    if os.path.isdir(_p) and _p not in sys.path:
        sys.path.insert(0, _p)

import numpy as np
import ml_dtypes

import concourse.bass as bass  # noqa: F401  (registers engine libraries)
import concourse.mybir as mybir
import concourse.tile as tile
from concourse import bacc
from concourse.bass_utils import run_bass_kernel_spmd

N_CORES = 8
M = 4          # ensemble members
B = 512        # batch
IN = 4096      # in_features (contraction)
OUT = 4096     # out_features
H = 32         # encoder hidden
P = 128        # partitions
KC = IN // P   # 32 contraction chunks of 128
KPG = 4        # kc per DMA group
G = KC // KPG  # 8 DMA groups
O_CORE = OUT // N_CORES   # 512 output columns per core
OC = O_CORE // P          # 4 o-chunks of 128 per core
N_WARM = 8     # PE warm-up matmuls (~3.4us cold => HAM unthrottled)

# cp16 column layout (bf16, [128, CP_W]): encoder matmul operands
CP_ALPHA = 0                  # [p, kc, m]   KC*M = 128
CP_ENC1 = CP_ALPHA + KC * M   # [p, kc, h]   KC*H = 1024
CP_W = CP_ENC1 + KC * H       # 1152

# cps33 column layout (f32, [33, SW33]); rows 0..31 = h
S_EPS = 0                     # epsT               M cols
S_ENCM = S_EPS + M            # encm_wT            H cols
S_ENCMB = S_ENCM + H          # encm_b             1
S_ENC1B = S_ENCMB + 1         # enc1_b             1
SW33 = S_ENC1B + 1

# gb32 column layout (f32, [128, GB_W])
GB_G = 0                      # [p, oc, m]  OC*M = 16
GB_B = GB_G + OC * M
GB_W = GB_B + OC * M          # 32

F32 = mybir.dt.float32
BF16 = mybir.dt.bfloat16
AF = mybir.ActivationFunctionType
ALU = mybir.AluOpType

_nc_cache = {}


def _build_nc():
    """Build and compile the per-core Bass/Tile program (SPMD, same on all 8)."""
    nc = bacc.Bacc("TRN2", num_devices=N_CORES, debug=False)

    xh_d = nc.declare_dram_parameter("xh", [P, KC, B], BF16, isOutput=False)
    wh_d = nc.declare_dram_parameter("wh", [P, KC, O_CORE], BF16, isOutput=False)
    cp16_d = nc.declare_dram_parameter("cp16", [P, CP_W], BF16, isOutput=False)
    dz16_d = nc.declare_dram_parameter("dz16", [H + 1, IN], BF16, isOutput=False)
    cps33_d = nc.declare_dram_parameter("cps33", [H, SW33], F32, isOutput=False)
    gb32_d = nc.declare_dram_parameter("gb32", [P, GB_W], F32, isOutput=False)
    out_d = nc.declare_dram_parameter("out", [O_CORE, M * B], F32, isOutput=True)

    with tile.TileContext(nc) as tc:
        with (
            tc.tile_pool(name="consts", bufs=1) as consts,
            tc.tile_pool(name="xt", bufs=G) as xt_pool,
            tc.tile_pool(name="wt", bufs=G) as wt_pool,
            tc.tile_pool(name="xa", bufs=6) as xa_pool,
            tc.tile_pool(name="xa3", bufs=KC) as xa3_pool,
            tc.tile_pool(name="ps", bufs=8, space="PSUM") as ps_pool,
            tc.tile_pool(name="osb", bufs=4) as out_pool,
        ):
            # ---- PE warm-up: garbage matmuls bridge the const-DMA latency
            # and trip the HAM activity monitor (1.2 -> 2.4 GHz).
            wu_src = consts.tile([P, B], BF16)
            nc.gpsimd.memset(wu_src[:], 0.0)
            # z_aug's ones row must beat the Pool-queue DMA issues (FIFO)
            z_aug = consts.tile([H + 1, M], BF16)
            nc.gpsimd.memset(z_aug[H : H + 1, :], 1.0)   # ones row for dec_b

            wu_ps = ps_pool.tile([P, B], F32, tag="ps")
            for i in range(N_WARM):
                nc.tensor.matmul(
                    wu_ps[:], lhsT=wu_src[:, :P], rhs=wu_src[:], start=True, stop=True
                )

            # ---- DMA issue.  All constants go on the ACT ring with the
            # full 16-engine bandwidth to themselves: the bulk x/w streams
            # (SP ring / Pool SWDGE ring) are explicitly held back behind
            # the last critical const so the encoder isn't starved by
            # packet-granularity round-robin against 8 MB of bulk traffic.
            cps33_sb = consts.tile([H, SW33], F32)
            nc.scalar.dma_start(cps33_sb[:], cps33_d.ap())
            cp16_sb = consts.tile([P, CP_W], BF16)
            nc.scalar.dma_start(cp16_sb[:], cp16_d.ap())
            dz16_sb = consts.tile([H + 1, IN], BF16)
            dz_dma = nc.scalar.dma_start(dz16_sb[:], dz16_d.ap())
            gb32_sb = consts.tile([P, GB_W], F32)
            nc.scalar.dma_start(gb32_sb[:], gb32_d.ap())
            xt_tiles = []
            wt_tiles = []
            for g in range(G):
                ks = slice(g * KPG, (g + 1) * KPG)
                xt = xt_pool.tile([P, KPG, B], BF16, tag="xt")
                xdma = nc.sync.dma_start(xt[:], xh_d.ap()[:, ks, :])
                wt = wt_pool.tile([P, KPG, O_CORE], BF16, tag="wt")
                wdma = nc.gpsimd.dma_start(wt[:], wh_d.ap()[:, ks, :])
                if g == 0:
                    tile.add_dep_helper(
                        wdma.ins, dz_dma.ins, reason="w stream after consts"
                    )
                xt_tiles.append(xt)
                wt_tiles.append(wt)

            # views into the packed constants
            alpha_v = cp16_sb[:, CP_ALPHA:CP_ENC1].rearrange("p (k m) -> p k m", m=M)
            enc1_v = cp16_sb[:, CP_ENC1:CP_W].rearrange("p (k h) -> p k h", h=H)
            decw_v = dz16_sb[:, :]                       # [33, IN] (row 32 = dec_b)
            eps_v = cps33_sb[:, S_EPS:S_ENCM]            # [32, M]
            encm_v = cps33_sb[:, S_ENCM:S_ENCMB]         # [32, H]
            encmb_v = cps33_sb[:, S_ENCMB:S_ENC1B]       # [32, 1]
            enc1b_v = cps33_sb[:, S_ENC1B:SW33]          # [32, 1]
            g_v = gb32_sb[:, GB_G:GB_B].rearrange("p (o m) -> p o m", m=M)
            b_v = gb32_sb[:, GB_B:GB_W].rearrange("p (o m) -> p o m", m=M)

            # ---- encoder (tiny)
            embT_ps = ps_pool.tile([H, M], F32, tag="ps")
            for k in range(KC):
                nc.tensor.matmul(
                    embT_ps[:],
                    lhsT=enc1_v[:, k, :],
                    rhs=alpha_v[:, k, :],
                    start=(k == 0),
                    stop=(k == KC - 1),
                )
            embT_sb = consts.tile([H, M], F32)
            nc.scalar.activation(embT_sb[:], embT_ps[:], AF.Relu, bias=enc1b_v)

            muT_ps = ps_pool.tile([H, M], F32, tag="ps")
            nc.tensor.matmul(
                muT_ps[:], lhsT=encm_v[:], rhs=embT_sb[:], start=True, stop=True
            )
            muT_sb = consts.tile([H, M], F32)
            nc.scalar.activation(muT_sb[:], muT_ps[:], AF.Identity, bias=encmb_v)

            eT_sb = consts.tile([H, M], F32)
            nc.scalar.activation(eT_sb[:], muT_sb[:], AF.Exp, scale=0.5)
            ez_sb = consts.tile([H, M], F32)
            nc.vector.tensor_mul(ez_sb[:], eT_sb[:], eps_v)
            nc.vector.tensor_add(z_aug[:H, :], ez_sb[:], muT_sb[:])

            # adecT: 32 independent matmuls (contraction 33, dec_b via ones row)
            # into one psum tile, then a single copy out.
            adec_ps = ps_pool.tile([P, KC * M], F32, tag="ps")
            for k in range(KC):
                nc.tensor.matmul(
                    adec_ps[:, k * M : (k + 1) * M],
                    lhsT=decw_v[:, k * P : (k + 1) * P],
                    rhs=z_aug[:],
                    start=True,
                    stop=True,
                )
            # the xa scale ops read adec straight from PSUM (region deps:
            # xa for chunk k only waits on adec matmul k, and DVE PSUM
            # access is no slower than SBUF)
            adecT_sb = adec_ps

            # consume the warm-up psum late so bacc DCE keeps the warm-up,
            # without blocking any queue at the head.
            wu_sink = consts.tile([P, B], F32)
            nc.vector.tensor_copy(wu_sink[:], wu_ps[:])

            # ---- main GEMM, models 0..2: k-major, 4 psum banks per model
            for m in range(M - 1):
                ps_tiles = [
                    ps_pool.tile([P, B], F32, tag="ps", name=f"mmps_{m}_{oc}")
                    for oc in range(OC)
                ]
                for k in range(KC):
                    g, j = divmod(k, KPG)
                    xa = xa_pool.tile([P, B], BF16, tag="xa")
                    nc.vector.tensor_scalar_mul(
                        xa[:], xt_tiles[g][:, j, :], adecT_sb[:, k * M + m : k * M + m + 1]
                    )
                    for oc in range(OC):
                        nc.tensor.matmul(
                            ps_tiles[oc][:],
                            lhsT=wt_tiles[g][:, j, oc * P : (oc + 1) * P],
                            rhs=xa[:],
                            start=(k == 0),
                            stop=(k == KC - 1),
                        )
                for oc in range(OC):
                    osb = out_pool.tile([P, B], F32, tag="osb")
                    nc.scalar.activation(
                        osb[:],
                        ps_tiles[oc][:],
                        AF.Identity,
                        bias=b_v[:, oc, m : m + 1],
                        scale=g_v[:, oc, m : m + 1],
                    )
                    nc.sync.dma_start(
                        out_d.ap()[oc * P : (oc + 1) * P, m * B : (m + 1) * B],
                        osb[:],
                    )

            # ---- last model: materialize all scaled activations up front,
            # then oc-major so each output chunk's epilogue + store overlaps
            # the remaining matmuls (shrinks the kernel tail).
            m = M - 1
            xa3_tiles = []
            for k in range(KC):
                xa3 = xa3_pool.tile([P, B], BF16, tag="xa3", name=f"xa3_{k}")
                nc.vector.tensor_scalar_mul(
                    xa3[:], xt_tiles[k // KPG][:, k % KPG, :],
                    adecT_sb[:, k * M + m : k * M + m + 1],
                )
                xa3_tiles.append(xa3)
            for oc in range(OC):
                ps3 = ps_pool.tile([P, B], F32, tag="ps", name=f"mmps3_{oc}")
                for k in range(KC):
                    g, j = divmod(k, KPG)
                    nc.tensor.matmul(
                        ps3[:],
                        lhsT=wt_tiles[g][:, j, oc * P : (oc + 1) * P],
                        rhs=xa3_tiles[k][:],
                        start=(k == 0),
                        stop=(k == KC - 1),
                    )
                osb3 = out_pool.tile([P, B], F32, tag="osb", name=f"osb3_{oc}")
                nc.scalar.activation(
                    osb3[:],
                    ps3[:],
                    AF.Identity,
                    bias=b_v[:, oc, m : m + 1],
                    scale=g_v[:, oc, m : m + 1],
                )
                nc.sync.dma_start(
                    out_d.ap()[oc * P : (oc + 1) * P, m * B : (m + 1) * B],
                    osb3[:],
                )

    nc.compile()
    return nc


def _get_nc():
    if "nc" not in _nc_cache:
        _nc_cache["nc"] = _build_nc()
    return _nc_cache["nc"]


def _pk(a2d):
    """(C*P, W) -> (P, C*W): row 128c+p -> [p, c, :] flattened."""
    c = a2d.shape[0] // P
    w = a2d.shape[1]
    return np.ascontiguousarray(
        a2d.reshape(c, P, w).transpose(1, 0, 2).reshape(P, c * w)
    )


def kernel(
    x, eps, alpha, gamma, bias_p, fc_w,
    enc1_w, enc1_b, encm_w, encm_b, dec_w, dec_b,
):
    bf16 = ml_dtypes.bfloat16
    f32 = np.float32
    asc = np.ascontiguousarray

    x = np.asarray(x, f32)
    fc_w = np.asarray(fc_w, f32)

    # x: (B, IN) -> xh (P, KC, B) bf16, xh[p,k,r] = x[r, 128k+p]
    xh = asc(x.astype(bf16).T.reshape(KC, P, B).transpose(1, 0, 2))
    # fc_w: (OUT, IN) -> per-core wh (P, KC, O_CORE) bf16
    wT_full = fc_w.astype(bf16).T  # (IN, OUT) view

    cp16 = np.empty((P, CP_W), bf16)
    cp16[:, CP_ALPHA:CP_ENC1] = _pk(asc(np.asarray(alpha, f32).T)).astype(bf16)
    cp16[:, CP_ENC1:CP_W] = _pk(asc(np.asarray(enc1_w, f32).T)).astype(bf16)

    dz16 = np.empty((H + 1, IN), bf16)
    dz16[:H] = np.asarray(dec_w, f32).T.astype(bf16)
    dz16[H] = np.asarray(dec_b, f32).astype(bf16)

    cps33 = np.zeros((H, SW33), f32)
    cps33[:, S_EPS:S_ENCM] = np.asarray(eps, f32).T
    cps33[:, S_ENCM:S_ENCMB] = np.asarray(encm_w, f32).T
    cps33[:, S_ENCMB] = np.asarray(encm_b, f32)
    cps33[:, S_ENC1B] = np.asarray(enc1_b, f32)

    gT_full = np.asarray(gamma, f32).T                    # (OUT, M)
    bT_full = np.asarray(bias_p, f32).T                   # (OUT, M)

    in_maps = []
    for c in range(N_CORES):
        o0, o1 = c * O_CORE, (c + 1) * O_CORE
        wh = asc(wT_full[:, o0:o1].reshape(KC, P, O_CORE).transpose(1, 0, 2))
        gb32 = np.empty((P, GB_W), f32)
        gb32[:, GB_G:GB_B] = _pk(asc(gT_full[o0:o1]))
        gb32[:, GB_B:GB_W] = _pk(asc(bT_full[o0:o1]))
        in_maps.append(
            {"xh": xh, "wh": wh, "cp16": cp16, "dz16": dz16,
             "cps33": cps33, "gb32": gb32}
        )

    nc = _get_nc()
    res = run_bass_kernel_spmd(nc, in_maps, list(range(N_CORES)))
    outT = np.concatenate(
        [res.results[c]["out"] for c in range(N_CORES)], axis=0
    )  # (OUT, M*B)
    return asc(outT.T.astype(np.float32))  # (M*B, OUT)


# revision 16
# speedup vs baseline: 1.0045x; 1.0045x over previous
"""Trainium2 Bass kernel for nn_Ensemble_FC (BatchEnsemble fully-connected layer).

Math (reference):
    emb   = relu(alpha @ enc1_w.T + enc1_b)          # (M, H)
    mu    = emb @ encm_w.T + encm_b                  # (M, H)
    z     = eps * exp(0.5 * mu) + mu
    adec  = z @ dec_w.T + dec_b                      # (M, IN)
    out[m*B+i, o] = (sum_k x[i,k] * adec[m,k] * fc_w[o,k]) * gamma[m,o] + bias_p[m,o]

Sharding: tensor-parallel column-split of fc_w / gamma / bias_p over
out_features (4096 -> 8 x 512).  Every core computes the tiny encoder
redundantly and the full (M*B = 2048)-row GEMM for its 512 output columns.

On-chip layout is transposed ([feature, row]) so per-model scales
(adec, gamma, bias) are per-partition scalars:
    out_core[o_local, m*B+i] = psum * gamma + bias,
    psum = sum_kc  wT[kc, o-chunk].T @ (xT[kc] * adecT[kc, m])
GEMM and encoder weights run in bf16 (rounded on host), fp32 PSUM
accumulation; the mu/exp/z chain and all epilogues are fp32.

Perf structure (trace-driven):
- ~7us fixed prologue (barriers + IRAM loads) before any instruction runs.
- Encoder constants go in small bf16 packs at the head of the ACT HWDGE
  ring so they don't queue behind the 8 MB bulk stream; gamma/bias ride a
  separate tiny f32 pack.  x stream on the SP ring, w stream on the Pool
  SWDGE ring — descriptor generation is ~0.6-1.0us of sequencer time per
  dma_start, so the three queues are issued in parallel.
- PE warm-up matmuls bridge the const-DMA latency and trip the HAM clock
  gate (cold PE runs at 1.2 GHz; >3.4us of sustained work => 2.4 GHz).
- dec_b is folded into the adec matmul via an augmented ones row; all 32
  adec chunk matmuls land in one PSUM tile, copied out once.
- The last model runs oc-major over pre-materialized scaled activations
  so the final epilogue + store overlaps the matmul stream.
"""

import os
import sys

for _p in ("/opt/trn_rl_repo",):
    if os.path.isdir(_p) and _p not in sys.path:
        sys.path.insert(0, _p)

import numpy as np
import ml_dtypes

import concourse.bass as bass  # noqa: F401  (registers engine libraries)
import concourse.mybir as mybir
import concourse.tile as tile
from concourse import bacc
from concourse.bass_utils import run_bass_kernel_spmd

N_CORES = 8
M = 4          # ensemble members
B = 512        # batch
IN = 4096      # in_features (contraction)
OUT = 4096     # out_features
H = 32         # encoder hidden
P = 128        # partitions
KC = IN // P   # 32 contraction chunks of 128
KPG = 4        # kc per DMA group
G = KC // KPG  # 8 DMA groups
O_CORE = OUT // N_CORES   # 512 output columns per core
OC = O_CORE // P          # 4 o-chunks of 128 per core
N_WARM = 8     # PE warm-up matmuls (~3.4us cold => HAM unthrottled)

# cp16 column layout (bf16, [128, CP_W]): encoder matmul operands
CP_ALPHA = 0                  # [p, kc, m]   KC*M = 128
CP_ENC1 = CP_ALPHA + KC * M   # [p, kc, h]   KC*H = 1024
CP_W = CP_ENC1 + KC * H       # 1152

# cps33 column layout (f32, [33, SW33]); rows 0..31 = h
S_EPS = 0                     # epsT               M cols
S_ENCM = S_EPS + M            # encm_wT            H cols
S_ENCMB = S_ENCM + H          # encm_b             1
S_ENC1B = S_ENCMB + 1         # enc1_b             1
SW33 = S_ENC1B + 1

# gb32 column layout (f32, [128, GB_W])
GB_G = 0                      # [p, oc, m]  OC*M = 16
GB_B = GB_G + OC * M
GB_W = GB_B + OC * M          # 32

F32 = mybir.dt.float32
BF16 = mybir.dt.bfloat16
AF = mybir.ActivationFunctionType
ALU = mybir.AluOpType

_nc_cache = {}


def _build_nc():
    """Build and compile the per-core Bass/Tile program (SPMD, same on all 8)."""
    nc = bacc.Bacc("TRN2", num_devices=N_CORES, debug=False)

    xh_d = nc.declare_dram_parameter("xh", [P, KC, B], BF16, isOutput=False)
    wh_d = nc.declare_dram_parameter("wh", [P, KC, O_CORE], BF16, isOutput=False)
    cp16_d = nc.declare_dram_parameter("cp16", [P, CP_W], BF16, isOutput=False)
    dz16_d = nc.declare_dram_parameter("dz16", [H + 1, IN], BF16, isOutput=False)
    cps33_d = nc.declare_dram_parameter("cps33", [H, SW33], F32, isOutput=False)
    gb32_d = nc.declare_dram_parameter("gb32", [P, GB_W], F32, isOutput=False)
    out_d = nc.declare_dram_parameter("out", [O_CORE, M * B], F32, isOutput=True)

    with tile.TileContext(nc) as tc:
        with (
            tc.tile_pool(name="consts", bufs=1) as consts,
            tc.tile_pool(name="xt", bufs=G) as xt_pool,
            tc.tile_pool(name="wt", bufs=G) as wt_pool,
            tc.tile_pool(name="xa", bufs=8) as xa_pool,
            tc.tile_pool(name="xa3", bufs=KC) as xa3_pool,
            tc.tile_pool(name="ps", bufs=8, space="PSUM") as ps_pool,
            tc.tile_pool(name="osb", bufs=4) as out_pool,
        ):
            # ---- PE warm-up: garbage matmuls bridge the const-DMA latency
            # and trip the HAM activity monitor (1.2 -> 2.4 GHz).
            wu_src = consts.tile([P, B], BF16)
            nc.gpsimd.memset(wu_src[:], 0.0)
            # z_aug's ones row must beat the Pool-queue DMA issues (FIFO)
            z_aug = consts.tile([H + 1, M], BF16)
            nc.gpsimd.memset(z_aug[H : H + 1, :], 1.0)   # ones row for dec_b

            wu_ps = ps_pool.tile([P, B], F32, tag="ps")
            for i in range(N_WARM):
                nc.tensor.matmul(
                    wu_ps[:], lhsT=wu_src[:, :P], rhs=wu_src[:], start=True, stop=True
                )

            # ---- DMA issue.  All constants go on the ACT ring with the
            # full 16-engine bandwidth to themselves: the bulk x/w streams
            # (SP ring / Pool SWDGE ring) are explicitly held back behind
            # the last critical const so the encoder isn't starved by
            # packet-granularity round-robin against 8 MB of bulk traffic.
            cps33_sb = consts.tile([H, SW33], F32)
            nc.scalar.dma_start(cps33_sb[:], cps33_d.ap())
            cp16_sb = consts.tile([P, CP_W], BF16)
            nc.scalar.dma_start(cp16_sb[:], cp16_d.ap())
            dz16_sb = consts.tile([H + 1, IN], BF16)
            dz_dma = nc.scalar.dma_start(dz16_sb[:], dz16_d.ap())
            gb32_sb = consts.tile([P, GB_W], F32)
            nc.scalar.dma_start(gb32_sb[:], gb32_d.ap())
            xt_tiles = []
            wt_tiles = []
            for g in range(G):
                ks = slice(g * KPG, (g + 1) * KPG)
                xt = xt_pool.tile([P, KPG, B], BF16, tag="xt")
                xdma = nc.sync.dma_start(xt[:], xh_d.ap()[:, ks, :])
                wt = wt_pool.tile([P, KPG, O_CORE], BF16, tag="wt")
                wdma = nc.gpsimd.dma_start(wt[:], wh_d.ap()[:, ks, :])
                if g == 0:
                    tile.add_dep_helper(
                        wdma.ins, dz_dma.ins, reason="w stream after consts"
                    )
                xt_tiles.append(xt)
                wt_tiles.append(wt)

            # views into the packed constants
            alpha_v = cp16_sb[:, CP_ALPHA:CP_ENC1].rearrange("p (k m) -> p k m", m=M)
            enc1_v = cp16_sb[:, CP_ENC1:CP_W].rearrange("p (k h) -> p k h", h=H)
            decw_v = dz16_sb[:, :]                       # [33, IN] (row 32 = dec_b)
            eps_v = cps33_sb[:, S_EPS:S_ENCM]            # [32, M]
            encm_v = cps33_sb[:, S_ENCM:S_ENCMB]         # [32, H]
            encmb_v = cps33_sb[:, S_ENCMB:S_ENC1B]       # [32, 1]
            enc1b_v = cps33_sb[:, S_ENC1B:SW33]          # [32, 1]
            g_v = gb32_sb[:, GB_G:GB_B].rearrange("p (o m) -> p o m", m=M)
            b_v = gb32_sb[:, GB_B:GB_W].rearrange("p (o m) -> p o m", m=M)

            # ---- encoder (tiny)
            embT_ps = ps_pool.tile([H, M], F32, tag="ps")
            for k in range(KC):
                nc.tensor.matmul(
                    embT_ps[:],
                    lhsT=enc1_v[:, k, :],
                    rhs=alpha_v[:, k, :],
                    start=(k == 0),
                    stop=(k == KC - 1),
                )
            embT_sb = consts.tile([H, M], F32)
            nc.scalar.activation(embT_sb[:], embT_ps[:], AF.Relu, bias=enc1b_v)

            muT_ps = ps_pool.tile([H, M], F32, tag="ps")
            nc.tensor.matmul(
                muT_ps[:], lhsT=encm_v[:], rhs=embT_sb[:], start=True, stop=True
            )
            muT_sb = consts.tile([H, M], F32)
            nc.scalar.activation(muT_sb[:], muT_ps[:], AF.Identity, bias=encmb_v)

            eT_sb = consts.tile([H, M], F32)
            nc.scalar.activation(eT_sb[:], muT_sb[:], AF.Exp, scale=0.5)
            ez_sb = consts.tile([H, M], F32)
            nc.vector.tensor_mul(ez_sb[:], eT_sb[:], eps_v)
            nc.vector.tensor_add(z_aug[:H, :], ez_sb[:], muT_sb[:])

            # adecT: 32 independent matmuls (contraction 33, dec_b via ones row)
            # into one psum tile, then a single copy out.
            adec_ps = ps_pool.tile([P, KC * M], F32, tag="ps")
            for k in range(KC):
                nc.tensor.matmul(
                    adec_ps[:, k * M : (k + 1) * M],
                    lhsT=decw_v[:, k * P : (k + 1) * P],
                    rhs=z_aug[:],
                    start=True,
                    stop=True,
                )
            # copy adec out of PSUM in 4 chunks so the first xa scale only
            # waits on the first 8 adec matmuls (pass A needs all 8 banks)
            adecT_sb = consts.tile([P, KC * M], F32)
            for q in range(4):
                cw = KC * M // 4
                nc.scalar.activation(
                    adecT_sb[:, q * cw : (q + 1) * cw],
                    adec_ps[:, q * cw : (q + 1) * cw],
                    AF.Copy,
                )

            # consume the warm-up psum late so bacc DCE keeps the warm-up,
            # without blocking any queue at the head.
            wu_sink = consts.tile([P, B], F32)
            nc.vector.tensor_copy(wu_sink[:], wu_ps[:])

            def epilogue(ps, oc, m, name):
                osb = out_pool.tile([P, B], F32, tag="osb", name=name)
                nc.scalar.activation(
                    osb[:],
                    ps[:],
                    AF.Identity,
                    bias=b_v[:, oc, m : m + 1],
                    scale=g_v[:, oc, m : m + 1],
                )
                nc.sync.dma_start(
                    out_d.ap()[oc * P : (oc + 1) * P, m * B : (m + 1) * B],
                    osb[:],
                )

            # ---- main GEMM.
            # Pass A: oc in {0,1} x all models, k-outer — each DMA group is
            # consumed over ~7us, so the matmul stream tracks the bulk-DMA
            # arrival rate without stalls while data is still landing.
            A_OCS = (0, 1)
            psA = {
                (oc, m): ps_pool.tile([P, B], F32, tag="ps", name=f"psA_{oc}_{m}")
                for oc in A_OCS
                for m in range(M)
            }
            for k in range(KC):
                g, j = divmod(k, KPG)
                for m in range(M):
                    xa = xa_pool.tile([P, B], BF16, tag="xa", name=f"xaA_{k}_{m}")
                    nc.vector.tensor_scalar_mul(
                        xa[:], xt_tiles[g][:, j, :],
                        adecT_sb[:, k * M + m : k * M + m + 1],
                    )
                    for oc in A_OCS:
                        nc.tensor.matmul(
                            psA[(oc, m)][:],
                            lhsT=wt_tiles[g][:, j, oc * P : (oc + 1) * P],
                            rhs=xa[:],
                            start=(k == 0),
                            stop=(k == KC - 1),
                        )
            for m in range(M):
                for oc in A_OCS:
                    epilogue(psA[(oc, m)], oc, m, f"osbA_{oc}_{m}")

            # Pass B: oc in {2,3}, group-major (all data resident by now);
            # per model the 32 scaled tiles are materialized once and used
            # by both oc chunks, and group completions stagger so the final
            # epilogue + store tail is tiny.
            B_OCS = (2, 3)
            for m in range(M):
                xab_tiles = []
                for k in range(KC):
                    xab = xa3_pool.tile(
                        [P, B], BF16, tag="xa3", name=f"xaB_{m}_{k}"
                    )
                    nc.vector.tensor_scalar_mul(
                        xab[:], xt_tiles[k // KPG][:, k % KPG, :],
                        adecT_sb[:, k * M + m : k * M + m + 1],
                    )
                    xab_tiles.append(xab)
                for oc in B_OCS:
                    psB = ps_pool.tile([P, B], F32, tag="ps", name=f"psB_{m}_{oc}")
                    for k in range(KC):
                        g, j = divmod(k, KPG)
                        nc.tensor.matmul(
                            psB[:],
                            lhsT=wt_tiles[g][:, j, oc * P : (oc + 1) * P],
                            rhs=xab_tiles[k][:],
                            start=(k == 0),
                            stop=(k == KC - 1),
                        )
                    epilogue(psB, oc, m, f"osbB_{m}_{oc}")

    nc.compile()
    return nc


def _get_nc():
    if "nc" not in _nc_cache:
        _nc_cache["nc"] = _build_nc()
    return _nc_cache["nc"]


def _pk(a2d):
    """(C*P, W) -> (P, C*W): row 128c+p -> [p, c, :] flattened."""
    c = a2d.shape[0] // P
    w = a2d.shape[1]
    return np.ascontiguousarray(
        a2d.reshape(c, P, w).transpose(1, 0, 2).reshape(P, c * w)
    )


def kernel(
    x, eps, alpha, gamma, bias_p, fc_w,
    enc1_w, enc1_b, encm_w, encm_b, dec_w, dec_b,
):
    bf16 = ml_dtypes.bfloat16
    f32 = np.float32
    asc = np.ascontiguousarray

    x = np.asarray(x, f32)
    fc_w = np.asarray(fc_w, f32)

    # x: (B, IN) -> xh (P, KC, B) bf16, xh[p,k,r] = x[r, 128k+p]
    xh = asc(x.astype(bf16).T.reshape(KC, P, B).transpose(1, 0, 2))
    # fc_w: (OUT, IN) -> per-core wh (P, KC, O_CORE) bf16
    wT_full = fc_w.astype(bf16).T  # (IN, OUT) view

    cp16 = np.empty((P, CP_W), bf16)
    cp16[:, CP_ALPHA:CP_ENC1] = _pk(asc(np.asarray(alpha, f32).T)).astype(bf16)
    cp16[:, CP_ENC1:CP_W] = _pk(asc(np.asarray(enc1_w, f32).T)).astype(bf16)

    dz16 = np.empty((H + 1, IN), bf16)
    dz16[:H] = np.asarray(dec_w, f32).T.astype(bf16)
    dz16[H] = np.asarray(dec_b, f32).astype(bf16)

    cps33 = np.zeros((H, SW33), f32)
    cps33[:, S_EPS:S_ENCM] = np.asarray(eps, f32).T
    cps33[:, S_ENCM:S_ENCMB] = np.asarray(encm_w, f32).T
    cps33[:, S_ENCMB] = np.asarray(encm_b, f32)
    cps33[:, S_ENC1B] = np.asarray(enc1_b, f32)

    gT_full = np.asarray(gamma, f32).T                    # (OUT, M)
    bT_full = np.asarray(bias_p, f32).T                   # (OUT, M)

    in_maps = []
    for c in range(N_CORES):
        o0, o1 = c * O_CORE, (c + 1) * O_CORE
        wh = asc(wT_full[:, o0:o1].reshape(KC, P, O_CORE).transpose(1, 0, 2))
        gb32 = np.empty((P, GB_W), f32)
        gb32[:, GB_G:GB_B] = _pk(asc(gT_full[o0:o1]))
        gb32[:, GB_B:GB_W] = _pk(asc(bT_full[o0:o1]))
        in_maps.append(
            {"xh": xh, "wh": wh, "cp16": cp16, "dz16": dz16,
             "cps33": cps33, "gb32": gb32}
        )

    nc = _get_nc()
    res = run_bass_kernel_spmd(nc, in_maps, list(range(N_CORES)))
    outT = np.concatenate(
        [res.results[c]["out"] for c in range(N_CORES)], axis=0
    )  # (OUT, M*B)
    return asc(outT.T.astype(np.float32))  # (M*B, OUT)


# revision 17
# speedup vs baseline: 1.0231x; 1.0185x over previous
"""Trainium2 Bass kernel for nn_Ensemble_FC (BatchEnsemble fully-connected layer).

Math (reference):
    emb   = relu(alpha @ enc1_w.T + enc1_b)          # (M, H)
    mu    = emb @ encm_w.T + encm_b                  # (M, H)
    z     = eps * exp(0.5 * mu) + mu
    adec  = z @ dec_w.T + dec_b                      # (M, IN)
    out[m*B+i, o] = (sum_k x[i,k] * adec[m,k] * fc_w[o,k]) * gamma[m,o] + bias_p[m,o]

Sharding: tensor-parallel column-split of fc_w / gamma / bias_p over
out_features (4096 -> 8 x 512).  Every core computes the tiny encoder
redundantly and the full (M*B = 2048)-row GEMM for its 512 output columns.

On-chip layout is transposed ([feature, row]) so per-model scales
(adec, gamma, bias) are per-partition scalars:
    out_core[o_local, m*B+i] = psum * gamma + bias,
    psum = sum_kc  wT[kc, o-chunk].T @ (xT[kc] * adecT[kc, m])
GEMM and encoder weights run in bf16 (rounded on host), fp32 PSUM
accumulation; the mu/exp/z chain and all epilogues are fp32.

Perf structure (trace-driven):
- ~7us fixed prologue (barriers + IRAM loads) before any instruction runs.
- Encoder constants go in small bf16 packs at the head of the ACT HWDGE
  ring so they don't queue behind the 8 MB bulk stream; gamma/bias ride a
  separate tiny f32 pack.  x stream on the SP ring, w stream on the Pool
  SWDGE ring — descriptor generation is ~0.6-1.0us of sequencer time per
  dma_start, so the three queues are issued in parallel.
- PE warm-up matmuls bridge the const-DMA latency and trip the HAM clock
  gate (cold PE runs at 1.2 GHz; >3.4us of sustained work => 2.4 GHz).
- dec_b is folded into the adec matmul via an augmented ones row; all 32
  adec chunk matmuls land in one PSUM tile, copied out once.
- The last model runs oc-major over pre-materialized scaled activations
  so the final epilogue + store overlaps the matmul stream.
"""

import os
import sys

for _p in ("/opt/trn_rl_repo",):
    if os.path.isdir(_p) and _p not in sys.path:
        sys.path.insert(0, _p)

import numpy as np
import ml_dtypes

import concourse.bass as bass  # noqa: F401  (registers engine libraries)
import concourse.mybir as mybir
import concourse.tile as tile
from concourse import bacc
from concourse.bass_utils import run_bass_kernel_spmd

N_CORES = 8
M = 4          # ensemble members
B = 512        # batch
IN = 4096      # in_features (contraction)
OUT = 4096     # out_features
H = 32         # encoder hidden
P = 128        # partitions
KC = IN // P   # 32 contraction chunks of 128
KPG = 4        # kc per DMA group
G = KC // KPG  # 8 DMA groups
O_CORE = OUT // N_CORES   # 512 output columns per core
OC = O_CORE // P          # 4 o-chunks of 128 per core
N_WARM = 8     # PE warm-up matmuls (~3.4us cold => HAM unthrottled)

# cp16 column layout (bf16, [128, CP_W]): encoder matmul operands
CP_ALPHA = 0                  # [p, kc, m]   KC*M = 128
CP_ENC1 = CP_ALPHA + KC * M   # [p, kc, h]   KC*H = 1024
CP_W = CP_ENC1 + KC * H       # 1152

# cps33 column layout (f32, [33, SW33]); rows 0..31 = h
S_EPS = 0                     # epsT               M cols
S_ENCM = S_EPS + M            # encm_wT            H cols
S_ENCMB = S_ENCM + H          # encm_b             1
S_ENC1B = S_ENCMB + 1         # enc1_b             1
SW33 = S_ENC1B + 1

# gb32 column layout (f32, [128, GB_W])
GB_G = 0                      # [p, oc, m]  OC*M = 16
GB_B = GB_G + OC * M
GB_W = GB_B + OC * M          # 32

F32 = mybir.dt.float32
BF16 = mybir.dt.bfloat16
AF = mybir.ActivationFunctionType
ALU = mybir.AluOpType

_nc_cache = {}


def _build_nc():
    """Build and compile the per-core Bass/Tile program (SPMD, same on all 8)."""
    nc = bacc.Bacc("TRN2", num_devices=N_CORES, debug=False)

    xh_d = nc.declare_dram_parameter("xh", [P, KC, B], BF16, isOutput=False)
    wh_d = nc.declare_dram_parameter("wh", [P, KC, O_CORE], BF16, isOutput=False)
    cp16_d = nc.declare_dram_parameter("cp16", [P, CP_W], BF16, isOutput=False)
    dz16_d = nc.declare_dram_parameter("dz16", [H + 1, IN], BF16, isOutput=False)
    cps33_d = nc.declare_dram_parameter("cps33", [H, SW33], F32, isOutput=False)
    gb32_d = nc.declare_dram_parameter("gb32", [P, GB_W], F32, isOutput=False)
    out_d = nc.declare_dram_parameter("out", [O_CORE, M * B], F32, isOutput=True)

    with tile.TileContext(nc) as tc:
        with (
            tc.tile_pool(name="consts", bufs=1) as consts,
            tc.tile_pool(name="xt", bufs=G) as xt_pool,
            tc.tile_pool(name="wt", bufs=G) as wt_pool,
            tc.tile_pool(name="xa", bufs=8) as xa_pool,
            tc.tile_pool(name="xa3", bufs=KC) as xa3_pool,
            tc.tile_pool(name="ps", bufs=8, space="PSUM") as ps_pool,
            tc.tile_pool(name="osb", bufs=4) as out_pool,
        ):
            # ---- PE warm-up: garbage matmuls bridge the const-DMA latency
            # and trip the HAM activity monitor (1.2 -> 2.4 GHz).
            wu_src = consts.tile([P, B], BF16)
            nc.gpsimd.memset(wu_src[:], 0.0)
            # z_aug's ones row must beat the Pool-queue DMA issues (FIFO)
            z_aug = consts.tile([H + 1, M], BF16)
            nc.gpsimd.memset(z_aug[H : H + 1, :], 1.0)   # ones row for dec_b

            wu_ps = ps_pool.tile([P, B], F32, tag="ps")
            for i in range(N_WARM):
                nc.tensor.matmul(
                    wu_ps[:], lhsT=wu_src[:, :P], rhs=wu_src[:], start=True, stop=True
                )

            # ---- DMA issue.  All constants go on the ACT ring with the
            # full 16-engine bandwidth to themselves: the bulk x/w streams
            # (SP ring / Pool SWDGE ring) are explicitly held back behind
            # the last critical const so the encoder isn't starved by
            # packet-granularity round-robin against 8 MB of bulk traffic.
            cps33_sb = consts.tile([H, SW33], F32)
            nc.scalar.dma_start(cps33_sb[:], cps33_d.ap())
            cp16_sb = consts.tile([P, CP_W], BF16)
            nc.scalar.dma_start(cp16_sb[:], cp16_d.ap())
            dz16_sb = consts.tile([H + 1, IN], BF16)
            dz_dma = nc.scalar.dma_start(dz16_sb[:], dz16_d.ap())
            gb32_sb = consts.tile([P, GB_W], F32)
            nc.scalar.dma_start(gb32_sb[:], gb32_d.ap())
            xt_tiles = []
            wt_tiles = []
            for g in range(G):
                ks = slice(g * KPG, (g + 1) * KPG)
                xt = xt_pool.tile([P, KPG, B], BF16, tag="xt")
                xdma = nc.sync.dma_start(xt[:], xh_d.ap()[:, ks, :])
                wt = wt_pool.tile([P, KPG, O_CORE], BF16, tag="wt")
                wdma = nc.gpsimd.dma_start(wt[:], wh_d.ap()[:, ks, :])
                if g == 0:
                    tile.add_dep_helper(
                        xdma.ins, dz_dma.ins, reason="x stream after consts"
                    )
                    tile.add_dep_helper(
                        wdma.ins, dz_dma.ins, reason="w stream after consts"
                    )
                xt_tiles.append(xt)
                wt_tiles.append(wt)

            # views into the packed constants
            alpha_v = cp16_sb[:, CP_ALPHA:CP_ENC1].rearrange("p (k m) -> p k m", m=M)
            enc1_v = cp16_sb[:, CP_ENC1:CP_W].rearrange("p (k h) -> p k h", h=H)
            decw_v = dz16_sb[:, :]                       # [33, IN] (row 32 = dec_b)
            eps_v = cps33_sb[:, S_EPS:S_ENCM]            # [32, M]
            encm_v = cps33_sb[:, S_ENCM:S_ENCMB]         # [32, H]
            encmb_v = cps33_sb[:, S_ENCMB:S_ENC1B]       # [32, 1]
            enc1b_v = cps33_sb[:, S_ENC1B:SW33]          # [32, 1]
            g_v = gb32_sb[:, GB_G:GB_B].rearrange("p (o m) -> p o m", m=M)
            b_v = gb32_sb[:, GB_B:GB_W].rearrange("p (o m) -> p o m", m=M)

            # ---- encoder (tiny)
            embT_ps = ps_pool.tile([H, M], F32, tag="ps")
            for k in range(KC):
                nc.tensor.matmul(
                    embT_ps[:],
                    lhsT=enc1_v[:, k, :],
                    rhs=alpha_v[:, k, :],
                    start=(k == 0),
                    stop=(k == KC - 1),
                )
            embT_sb = consts.tile([H, M], F32)
            nc.scalar.activation(embT_sb[:], embT_ps[:], AF.Relu, bias=enc1b_v)

            muT_ps = ps_pool.tile([H, M], F32, tag="ps")
            nc.tensor.matmul(
                muT_ps[:], lhsT=encm_v[:], rhs=embT_sb[:], start=True, stop=True
            )
            muT_sb = consts.tile([H, M], F32)
            nc.scalar.activation(muT_sb[:], muT_ps[:], AF.Identity, bias=encmb_v)

            eT_sb = consts.tile([H, M], F32)
            nc.scalar.activation(eT_sb[:], muT_sb[:], AF.Exp, scale=0.5)
            ez_sb = consts.tile([H, M], F32)
            nc.vector.tensor_mul(ez_sb[:], eT_sb[:], eps_v)
            nc.vector.tensor_add(z_aug[:H, :], ez_sb[:], muT_sb[:])

            # adecT: 32 independent matmuls (contraction 33, dec_b via ones row)
            # into one psum tile, then a single copy out.
            adec_ps = ps_pool.tile([P, KC * M], F32, tag="ps")
            for k in range(KC):
                nc.tensor.matmul(
                    adec_ps[:, k * M : (k + 1) * M],
                    lhsT=decw_v[:, k * P : (k + 1) * P],
                    rhs=z_aug[:],
                    start=True,
                    stop=True,
                )
            # copy adec out of PSUM in 4 chunks so the first xa scale only
            # waits on the first 8 adec matmuls (pass A needs all 8 banks)
            adecT_sb = consts.tile([P, KC * M], F32)
            for q in range(4):
                cw = KC * M // 4
                nc.scalar.activation(
                    adecT_sb[:, q * cw : (q + 1) * cw],
                    adec_ps[:, q * cw : (q + 1) * cw],
                    AF.Copy,
                )

            # consume the warm-up psum late so bacc DCE keeps the warm-up,
            # without blocking any queue at the head.
            wu_sink = consts.tile([P, B], F32)
            nc.vector.tensor_copy(wu_sink[:], wu_ps[:])

            def epilogue(ps, oc, m, name):
                osb = out_pool.tile([P, B], F32, tag="osb", name=name)
                nc.scalar.activation(
                    osb[:],
                    ps[:],
                    AF.Identity,
                    bias=b_v[:, oc, m : m + 1],
                    scale=g_v[:, oc, m : m + 1],
                )
                nc.sync.dma_start(
                    out_d.ap()[oc * P : (oc + 1) * P, m * B : (m + 1) * B],
                    osb[:],
                )

            # ---- main GEMM.
            # Pass A: oc in {0,1} x all models, k-outer — each DMA group is
            # consumed over ~7us, so the matmul stream tracks the bulk-DMA
            # arrival rate without stalls while data is still landing.
            A_OCS = (0, 1)
            psA = {
                (oc, m): ps_pool.tile([P, B], F32, tag="ps", name=f"psA_{oc}_{m}")
                for oc in A_OCS
                for m in range(M)
            }
            for k in range(KC):
                g, j = divmod(k, KPG)
                for m in range(M):
                    xa = xa_pool.tile([P, B], BF16, tag="xa", name=f"xaA_{k}_{m}")
                    nc.vector.tensor_scalar_mul(
                        xa[:], xt_tiles[g][:, j, :],
                        adecT_sb[:, k * M + m : k * M + m + 1],
                    )
                    for oc in A_OCS:
                        nc.tensor.matmul(
                            psA[(oc, m)][:],
                            lhsT=wt_tiles[g][:, j, oc * P : (oc + 1) * P],
                            rhs=xa[:],
                            start=(k == 0),
                            stop=(k == KC - 1),
                        )
            for m in range(M):
                for oc in A_OCS:
                    epilogue(psA[(oc, m)], oc, m, f"osbA_{oc}_{m}")

            # Pass B: oc in {2,3}, group-major (all data resident by now);
            # per model the 32 scaled tiles are materialized once and used
            # by both oc chunks, and group completions stagger so the final
            # epilogue + store tail is tiny.
            B_OCS = (2, 3)
            for m in range(M):
                xab_tiles = []
                for k in range(KC):
                    xab = xa3_pool.tile(
                        [P, B], BF16, tag="xa3", name=f"xaB_{m}_{k}"
                    )
                    nc.vector.tensor_scalar_mul(
                        xab[:], xt_tiles[k // KPG][:, k % KPG, :],
                        adecT_sb[:, k * M + m : k * M + m + 1],
                    )
                    xab_tiles.append(xab)
                for oc in B_OCS:
                    psB = ps_pool.tile([P, B], F32, tag="ps", name=f"psB_{m}_{oc}")
                    for k in range(KC):
                        g, j = divmod(k, KPG)
                        nc.tensor.matmul(
                            psB[:],
                            lhsT=wt_tiles[g][:, j, oc * P : (oc + 1) * P],
                            rhs=xab_tiles[k][:],
                            start=(k == 0),
                            stop=(k == KC - 1),
                        )
                    epilogue(psB, oc, m, f"osbB_{m}_{oc}")

    nc.compile()
    return nc


def _get_nc():
    if "nc" not in _nc_cache:
        _nc_cache["nc"] = _build_nc()
    return _nc_cache["nc"]


def _pk(a2d):
    """(C*P, W) -> (P, C*W): row 128c+p -> [p, c, :] flattened."""
    c = a2d.shape[0] // P
    w = a2d.shape[1]
    return np.ascontiguousarray(
        a2d.reshape(c, P, w).transpose(1, 0, 2).reshape(P, c * w)
    )


def kernel(
    x, eps, alpha, gamma, bias_p, fc_w,
    enc1_w, enc1_b, encm_w, encm_b, dec_w, dec_b,
):
    bf16 = ml_dtypes.bfloat16
    f32 = np.float32
    asc = np.ascontiguousarray

    x = np.asarray(x, f32)
    fc_w = np.asarray(fc_w, f32)

    # x: (B, IN) -> xh (P, KC, B) bf16, xh[p,k,r] = x[r, 128k+p]
    xh = asc(x.astype(bf16).T.reshape(KC, P, B).transpose(1, 0, 2))
    # fc_w: (OUT, IN) -> per-core wh (P, KC, O_CORE) bf16
    wT_full = fc_w.astype(bf16).T  # (IN, OUT) view

    cp16 = np.empty((P, CP_W), bf16)
    cp16[:, CP_ALPHA:CP_ENC1] = _pk(asc(np.asarray(alpha, f32).T)).astype(bf16)
    cp16[:, CP_ENC1:CP_W] = _pk(asc(np.asarray(enc1_w, f32).T)).astype(bf16)

    dz16 = np.empty((H + 1, IN), bf16)
    dz16[:H] = np.asarray(dec_w, f32).T.astype(bf16)
    dz16[H] = np.asarray(dec_b, f32).astype(bf16)

    cps33 = np.zeros((H, SW33), f32)
    cps33[:, S_EPS:S_ENCM] = np.asarray(eps, f32).T
    cps33[:, S_ENCM:S_ENCMB] = np.asarray(encm_w, f32).T
    cps33[:, S_ENCMB] = np.asarray(encm_b, f32)
    cps33[:, S_ENC1B] = np.asarray(enc1_b, f32)

    gT_full = np.asarray(gamma, f32).T                    # (OUT, M)
    bT_full = np.asarray(bias_p, f32).T                   # (OUT, M)

    in_maps = []
    for c in range(N_CORES):
        o0, o1 = c * O_CORE, (c + 1) * O_CORE
        wh = asc(wT_full[:, o0:o1].reshape(KC, P, O_CORE).transpose(1, 0, 2))
        gb32 = np.empty((P, GB_W), f32)
        gb32[:, GB_G:GB_B] = _pk(asc(gT_full[o0:o1]))
        gb32[:, GB_B:GB_W] = _pk(asc(bT_full[o0:o1]))
        in_maps.append(
            {"xh": xh, "wh": wh, "cp16": cp16, "dz16": dz16,
             "cps33": cps33, "gb32": gb32}
        )

    nc = _get_nc()
    res = run_bass_kernel_spmd(nc, in_maps, list(range(N_CORES)))
    outT = np.concatenate(
        [res.results[c]["out"] for c in range(N_CORES)], axis=0
    )  # (OUT, M*B)
    return asc(outT.T.astype(np.float32))  # (M*B, OUT)
